# revision 21
# baseline (speedup 1.0000x reference)
"""Multi-head attention (B=2, N=2048, D=1024, H=16, hd=64) on 8 trn2 NeuronCores.

Sharding: 8 cores = 2 (batch) x 4 (head groups of 4 heads).
Core c: batch b = c // 4, heads hg*4 .. hg*4+3 where hg = c % 4.

Per-core program (identical SPMD program, per-core data):
  inputs (DRAM):
    xT     [1024, 2048]  = x[b].T
    wqkT   [1024, 512]   = w_qkv[[q rows, k rows] of local heads].T
    wvT    [1024, 256]   = w_qkv[v rows of local heads].T
    wprojT [256, 1024]   = w_proj[:, local head cols].T
    ident  [128, 128]    = identity (for PE transposes)
  output:
    out    [2048, 1024]  partial (row-parallel) projection output

Pipeline design (PE p-state requires gap-free streaming; the exp on ACT is
the per-chain cadence):
  - scores_T [keys, q] = kT.T @ qT (2 matmuls / key-tile-pair), exp on ACT
    (scale 1/8 fused) -> probs bf16.
  - Flipped PV: pv[q, 65] += probs_qsub.T @ v_aug per q-subtile of 128 with
    probs as the matmul stationary (65-column streams, psum memset-zeroed,
    start=False so no bank-wide psum zeroing). Column 64 accumulates the
    softmax denominator via the ones-augmented V.
  - DVE reciprocal of the denominators + per-partition tensor_scalar
    multiply normalizes; PE transpose (identity matmul) restores [hd, q]
    for the projection lhsT layout.
  - All qkv/proj gemm work not needed up-front (k h23, q for later
    q-blocks, all v tiles, projection) is held in a filler queue of
    generators and pumped between scores and PV inside each chain step so
    the PE never idles while ACT exps.

Host unshard: out[b] = sum over 4 head-group partials + b_proj.
"""

import sys

if "/opt/trn_rl_repo" not in sys.path:
    sys.path.insert(0, "/opt/trn_rl_repo")

from collections import deque

import numpy as np

B, N, D, H, HD = 2, 2048, 1024, 16, 64
NCORES = 8
HPC = 4               # heads per core
LQK = HPC * HD        # 256 local q (or k) rows
SCALE = HD ** -0.5    # 0.125

_CACHE = {}


def _build_program():
    import concourse.tile as tile
    from concourse import bacc, mybir

    F32 = mybir.dt.float32
    BF16 = mybir.dt.bfloat16
    Exp = mybir.ActivationFunctionType.Exp
    Mult = mybir.AluOpType.mult

    nc = bacc.Bacc("TRN2", target_bir_lowering=False, debug=False,
                   num_devices=NCORES)

    xT_d = nc.dram_tensor("xT", [D, N], BF16, kind="ExternalInput").ap()
    wqkT_d = nc.dram_tensor("wqkT", [D, 2 * LQK], BF16, kind="ExternalInput").ap()
    wvT_d = nc.dram_tensor("wvT", [D, LQK], BF16, kind="ExternalInput").ap()
    wprojT_d = nc.dram_tensor("wprojT", [LQK, D], BF16, kind="ExternalInput").ap()
    ident_d = nc.dram_tensor("ident", [128, 128], BF16, kind="ExternalInput").ap()
    out_d = nc.dram_tensor("out", [N, D], BF16, kind="ExternalOutput").ap()

    KT = D // 128        # 8 contraction tiles for qkv gemms
    NB = N // 512        # 4 seq blocks
    NT = N // 128        # 16 seq tiles

    with tile.TileContext(nc) as tc:
        with (
            nc.allow_low_precision(reason="bf16 matmul operands"),
            tc.tile_pool(name="const", bufs=1) as cpool,
            tc.tile_pool(name="w", bufs=1) as wpool,
            tc.tile_pool(name="x", bufs=1) as xpool,
            tc.tile_pool(name="qk", bufs=1) as qkpool,
            tc.tile_pool(name="vaug", bufs=1) as vapool,
            tc.tile_pool(name="ao", bufs=1) as aopool,
            tc.tile_pool(name="probs", bufs=18) as prpool,
            tc.tile_pool(name="small", bufs=2) as smpool,
            tc.tile_pool(name="psbig", bufs=2, space="PSUM") as psbig,
            tc.tile_pool(name="pspv", bufs=2, space="PSUM") as pspv,
            tc.tile_pool(name="pstr", bufs=1, space="PSUM") as pstr,
            tc.tile_pool(name="psgp", bufs=1, space="PSUM") as psgp,
        ):
            ones_f32 = cpool.tile([128, 128], F32)
            nc.vector.memset(ones_f32[:, :], 1.0)
            ident_sb = cpool.tile([128, 128], BF16)
            nc.sync.dma_start(out=ident_sb[:, :], in_=ident_d[:, :])

            # ---- input DMAs (kt-chunked so compute starts early) ----
            x_sb = xpool.tile([128, KT, N], BF16)
            wqk_sb = wpool.tile([128, KT, 2 * LQK], BF16)
            wv_sb = wpool.tile([128, KT, LQK], BF16)
            xT_r = xT_d.rearrange("(kt p) n -> p kt n", p=128)
            wqkT_r = wqkT_d.rearrange("(kt p) m -> p kt m", p=128)
            wvT_r = wvT_d.rearrange("(kt p) m -> p kt m", p=128)
            for kt in range(KT):
                nc.sync.dma_start(out=wqk_sb[:, kt, :], in_=wqkT_r[:, kt, :])
                for half in range(2):
                    nc.sync.dma_start(
                        out=x_sb[:, kt, half * 1024:(half + 1) * 1024],
                        in_=xT_r[:, kt, half * 1024:(half + 1) * 1024])
            for kt in range(KT):
                nc.sync.dma_start(out=wv_sb[:, kt, :], in_=wvT_r[:, kt, :])
            wproj_sb = wpool.tile([128, 2, D], BF16)
            nc.sync.dma_start(
                out=wproj_sb[:, :, :],
                in_=wprojT_d.rearrange("(kt p) o -> p kt o", p=128))

            # qk_sb m-tile layout: m=0: q heads 0,1 / m=1: q heads 2,3
            #                      m=2: k heads 0,1 / m=3: k heads 2,3
            qk_sb = qkpool.tile([128, 4, N], BF16)
            v_sb = vapool.tile([128, NT, HPC, HD + 1], BF16)
            ao_sb = aopool.tile([128, 2, N], BF16)

            # ---- filler generators: one matmul per yield ----
            def gen_qk(m, nb):
                wm = 0 if m < 2 else LQK          # q cols 0..255, k cols 256..511
                wcol = wm + (m % 2) * 128
                gp = psgp.tile([128, 512], F32, tag="gp")
                for kt in range(KT):
                    nc.tensor.matmul(
                        gp[:, :],
                        wqk_sb[:, kt, wcol:wcol + 128],
                        x_sb[:, kt, nb * 512:(nb + 1) * 512],
                        start=(kt == 0), stop=(kt == KT - 1),
                    )
                    yield
                nc.vector.tensor_copy(
                    qk_sb[:, m, nb * 512:(nb + 1) * 512], gp[:, :])

            def gen_v(st):
                gp = psgp.tile([128, 512], F32, tag="gp")
                for kt in range(KT):
                    nc.tensor.matmul(
                        gp[:, 0:LQK],
                        x_sb[:, kt, st * 128:(st + 1) * 128],
                        wv_sb[:, kt, :],
                        start=(kt == 0), stop=(kt == KT - 1),
                    )
                    yield
                nc.vector.tensor_copy(
                    v_sb[:, st, :, 0:HD],
                    gp[:, 0:LQK].rearrange("p (h d) -> p h d", h=HPC))
                nc.vector.tensor_copy(
                    v_sb[:, st, :, HD:HD + 1],
                    ones_f32[:, 0:HPC].rearrange("p (h c) -> p h c", c=1))

            outst_by_nt = {}

            def gen_proj(nt, ob):
                gp = psgp.tile([128, 512], F32, tag="gp")
                for kt2 in range(2):
                    nc.tensor.matmul(
                        gp[:, :],
                        ao_sb[:, kt2, nt * 128:(nt + 1) * 128],
                        wproj_sb[:, kt2, ob * 512:(ob + 1) * 512],
                        start=(kt2 == 0), stop=(kt2 == 1),
                    )
                    yield
                if ob == 0:
                    outst = smpool.tile([128, 1024], BF16, tag="outst")
                    outst_by_nt[nt] = outst
                else:
                    outst = outst_by_nt.pop(nt)
                nc.vector.tensor_copy(outst[:, ob * 512:(ob + 1) * 512], gp[:, :])
                if ob == 1:
                    nc.sync.dma_start(out=out_d[nt * 128:(nt + 1) * 128, :],
                                      in_=outst[:, :])

            queue = deque()

            def pump(n):
                while n > 0 and queue:
                    try:
                        next(queue[0])
                        n -= 1
                    except StopIteration:
                        queue.popleft()

            def pump_all():
                while queue:
                    pump(1 << 30)

            # ---- attention primitives (software-pipelined chains) ----
            def scores_one(h, qb, kk):
                """2 score matmuls + exp for key-tile pair kk; returns probs."""
                pi = (h % 2) * 64
                mq, mk = h // 2, 2 + h // 2
                qT = qk_sb[pi:pi + 64, mq, qb * 512:(qb + 1) * 512]
                sc = psbig.tile([128, 1024], F32, tag="big")
                pr = prpool.tile([128, 1024], BF16, tag="probs")
                for j in range(2):
                    kt = 2 * kk + j
                    kT = qk_sb[pi:pi + 64, mk, kt * 128:(kt + 1) * 128]
                    nc.tensor.matmul(
                        sc[:, j * 512:(j + 1) * 512], kT, qT,
                        start=True, stop=True)
                nc.scalar.activation(pr[:, :], sc[:, :], Exp, scale=SCALE)
                return pr

            def pvs_one(pv, h, kk, pr):
                for qs in range(4):
                    for j in range(2):
                        kt = 2 * kk + j
                        nc.tensor.matmul(
                            pv[:, qs, 0:HD + 1],
                            pr[:, j * 512 + qs * 128:j * 512 + (qs + 1) * 128],
                            v_sb[:, kt, h, :],
                            start=False,
                            stop=(kk == KT - 1 and j == 1),
                            skip_group_check=True,
                        )

            def norm_tail(h, qb, pv):
                """DVE: recip of denominators + per-partition normalize to
                bf16; PE: transpose back to [hd, q]; DVE: copy into ao_sb."""
                pi = (h % 2) * 64
                den = smpool.tile([128, 4], F32, tag="den")
                nc.vector.tensor_copy(den[:, :], pv[:, :, HD])
                rec = smpool.tile([128, 4], F32, tag="rec")
                nc.vector.reciprocal(rec[:, :], den[:, :])
                aos = smpool.tile([128, 4, HD], BF16, tag="aos")
                for qs in range(4):
                    nc.vector.tensor_scalar(
                        aos[:, qs, :], pv[:, qs, 0:HD], rec[:, qs:qs + 1],
                        None, Mult)
                pt = pstr.tile([64, 1024], BF16, tag="tr")
                for qs in range(4):
                    nc.tensor.matmul(
                        pt[:, qs * 128:(qs + 1) * 128], aos[:, qs, :],
                        ident_sb[:, :], is_transpose=True)
                nc.vector.tensor_copy(
                    ao_sb[pi:pi + 64, h // 2, qb * 512:(qb + 1) * 512],
                    pt[:, 0:512])

            # ---- schedule ----
            # prelude emitted directly through the (still idle) scores pool
            # with double buffering: k h0/h1 (all nb), q h0/h1 for qb0, v st0/1
            def pre_qk(ps, half, m, nb):
                wm = 0 if m < 2 else LQK
                wcol = wm + (m % 2) * 128
                for kt in range(KT):
                    nc.tensor.matmul(
                        ps[:, half * 512:(half + 1) * 512],
                        wqk_sb[:, kt, wcol:wcol + 128],
                        x_sb[:, kt, nb * 512:(nb + 1) * 512],
                        start=(kt == 0), stop=(kt == KT - 1),
                    )

            for nn in range(2):             # k h0/h1: nb pairs (0,1) and (2,3)
                ps = psbig.tile([128, 1024], F32, tag="big")
                for half in range(2):
                    pre_qk(ps, half, 2, nn * 2 + half)
                nc.vector.tensor_copy(
                    qk_sb[:, 2, nn * 1024:(nn + 1) * 1024], ps[:, :])
            ps = psbig.tile([128, 1024], F32, tag="big")
            pre_qk(ps, 0, 0, 0)             # q h0/h1 for qb0
            for kt in range(KT):            # v st0 into the second bank
                nc.tensor.matmul(
                    ps[:, 512:512 + LQK],
                    x_sb[:, kt, 0:128], wv_sb[:, kt, :],
                    start=(kt == 0), stop=(kt == KT - 1),
                )
            nc.vector.tensor_copy(qk_sb[:, 0, 0:512], ps[:, 0:512])
            nc.vector.tensor_copy(
                v_sb[:, 0, :, 0:HD],
                ps[:, 512:512 + LQK].rearrange("p (h d) -> p h d", h=HPC))
            nc.vector.tensor_copy(
                v_sb[:, 0, :, HD:HD + 1],
                ones_f32[:, 0:HPC].rearrange("p (h c) -> p h c", c=1))
            ps = psbig.tile([128, 1024], F32, tag="big")
            for kt in range(KT):            # v st1
                nc.tensor.matmul(
                    ps[:, 0:LQK],
                    x_sb[:, kt, 128:256], wv_sb[:, kt, :],
                    start=(kt == 0), stop=(kt == KT - 1),
                )
            nc.vector.tensor_copy(
                v_sb[:, 1, :, 0:HD],
                ps[:, 0:LQK].rearrange("p (h d) -> p h d", h=HPC))
            nc.vector.tensor_copy(
                v_sb[:, 1, :, HD:HD + 1],
                ones_f32[:, 0:HPC].rearrange("p (h c) -> p h c", c=1))

            # v must be complete before any PV consumes it (JIT v copies
            # race same-chain PV reads): all v gens are pumped during chain
            # 0's scores phase, front-loaded so the last copy lands a full
            # key-pair before the first PV.
            queue.extend([gen_v(st) for st in range(2, NT)])
            queue.append(gen_qk(1, 0))                          # q h2/h3 qb0
            queue.extend([gen_qk(3, nb) for nb in range(NB)])   # k h2/h3
            for nb in range(1, NB):
                queue.append(gen_qk(0, nb))
                queue.append(gen_qk(1, nb))

            chains = [(h, qb) for qb in range(NB) for h in range(HPC)]
            NC = len(chains)

            # chain 0 scores phase, v tiles as filler (front-loaded)
            prs = {0: []}
            for kk in range(KT):
                prs[0].append(scores_one(0, 0, kk))
                pump(16 if kk < KT - 1 else 0)

            # steady state: chain c's PVs interleaved with chain c+1's
            # scores/exps so ACT has no chain-boundary bubble
            for c in range(NC):
                h, qb = chains[c]
                nxt = chains[c + 1] if c + 1 < NC else None
                pv = pspv.tile([128, 4, 128], F32, tag="pv")
                nc.vector.memset(pv[:, :, :], 0.0)  # full-bank tile
                if nxt is not None and nxt[1] != qb:
                    pump(8)     # flush the next q-block's q tiles
                if nxt is not None:
                    prs[c + 1] = []
                # chain 0 must flush q(h23)@qb0 + all k(h23) (40 matmuls)
                # before chain 2's scores prefetch inside chain 1's PV phase;
                # 3rd chain of a q-block pre-pumps the boundary backlog
                fill = 5 if c == 0 else (3 if (c % HPC == 2) else 2)
                for kk in range(KT):
                    pvs_one(pv, h, kk, prs[c][kk])
                    if nxt is not None:
                        prs[c + 1].append(scores_one(nxt[0], nxt[1], kk))
                    pump(fill)
                del prs[c]
                norm_tail(h, qb, pv)
                if c % HPC == HPC - 1 and qb < NB - 1:
                    for nt in range(qb * 4, qb * 4 + 4):
                        queue.append(gen_proj(nt, 0))
                        queue.append(gen_proj(nt, 1))

            pump_all()
            for nt in range(12, 16):
                ps = psbig.tile([128, 1024], F32, tag="big")
                for ob in range(2):
                    for kt2 in range(2):
                        nc.tensor.matmul(
                            ps[:, ob * 512:(ob + 1) * 512],
                            ao_sb[:, kt2, nt * 128:(nt + 1) * 128],
                            wproj_sb[:, kt2, ob * 512:(ob + 1) * 512],
                            start=(kt2 == 0), stop=(kt2 == 1),
                        )
                outst = smpool.tile([128, D], BF16, tag="outbig")
                nc.vector.tensor_copy(outst[:, :], ps[:, :])
                nc.sync.dma_start(out=out_d[nt * 128:(nt + 1) * 128, :],
                                  in_=outst[:, :])

    nc.compile()
    return nc


def _get_program():
    if "nc" not in _CACHE:
        _CACHE["nc"] = _build_program()
    return _CACHE["nc"]


def _make_in_maps(x, w_qkv, w_proj):
    import ml_dtypes
    bf16 = ml_dtypes.bfloat16
    x = np.asarray(x, dtype=np.float32)
    w_qkv = np.asarray(w_qkv, dtype=np.float32)
    w_proj = np.asarray(w_proj, dtype=np.float32)
    ident = np.eye(128, dtype=np.float32).astype(bf16)
    xT = [np.ascontiguousarray(x[b].T).astype(bf16) for b in range(B)]
    in_maps = []
    for c in range(NCORES):
        b, hg = c // 4, c % 4
        rows = slice(hg * LQK, (hg + 1) * LQK)
        qk_rows = np.r_[np.arange(hg * LQK, (hg + 1) * LQK),
                        D + np.arange(hg * LQK, (hg + 1) * LQK)]
        in_maps.append({
            "xT": xT[b],
            "wqkT": np.ascontiguousarray(w_qkv[qk_rows, :].T).astype(bf16),
            "wvT": np.ascontiguousarray(
                w_qkv[2 * D + np.arange(hg * LQK, (hg + 1) * LQK), :].T).astype(bf16),
            "wprojT": np.ascontiguousarray(w_proj[:, rows].T).astype(bf16),
            "ident": ident,
        })
    return in_maps


def kernel(x, w_qkv, w_proj, b_proj, _return_results=False, _trace=False):
    from concourse import bass_utils

    nc = _get_program()
    in_maps = _make_in_maps(x, w_qkv, w_proj)
    res = bass_utils.run_bass_kernel_spmd(
        nc, in_maps, list(range(NCORES)), trace=_trace)
    partials = np.stack([res.results[c]["out"] for c in range(NCORES)])
    out = partials.reshape(B, 4, N, D).sum(axis=1, dtype=np.float32)
    out = out + np.asarray(b_proj, dtype=np.float32)[None, None, :]
    out = out.astype(np.float32)
    if _return_results:
        return out, res
    return out


# revision 22
# speedup vs baseline: 1.0028x; 1.0028x over previous
"""Multi-head attention (B=2, N=2048, D=1024, H=16, hd=64) on 8 trn2 NeuronCores.

Sharding: 8 cores = 2 (batch) x 4 (head groups of 4 heads).
Core c: batch b = c // 4, heads hg*4 .. hg*4+3 where hg = c % 4.

Per-core program (identical SPMD program, per-core data):
  inputs (DRAM):
    xT     [1024, 2048]  = x[b].T
    wqkT   [1024, 512]   = w_qkv[[q rows, k rows] of local heads].T
    wvT    [1024, 256]   = w_qkv[v rows of local heads].T
    wprojT [256, 1024]   = w_proj[:, local head cols].T
    ident  [128, 128]    = identity (for PE transposes)
  output:
    out    [2048, 1024]  partial (row-parallel) projection output

Pipeline design (PE p-state requires gap-free streaming; the exp on ACT is
the per-chain cadence):
  - scores_T [keys, q] = kT.T @ qT (2 matmuls / key-tile-pair), exp on ACT
    (scale 1/8 fused) -> probs bf16.
  - Flipped PV: pv[q, 65] += probs_qsub.T @ v_aug per q-subtile of 128 with
    probs as the matmul stationary (65-column streams, psum memset-zeroed,
    start=False so no bank-wide psum zeroing). Column 64 accumulates the
    softmax denominator via the ones-augmented V.
  - DVE reciprocal of the denominators + per-partition tensor_scalar
    multiply normalizes; PE transpose (identity matmul) restores [hd, q]
    for the projection lhsT layout.
  - All qkv/proj gemm work not needed up-front (k h23, q for later
    q-blocks, all v tiles, projection) is held in a filler queue of
    generators and pumped between scores and PV inside each chain step so
    the PE never idles while ACT exps.

Host unshard: out[b] = sum over 4 head-group partials + b_proj.
"""

import sys

if "/opt/trn_rl_repo" not in sys.path:
    sys.path.insert(0, "/opt/trn_rl_repo")

from collections import deque

import numpy as np

B, N, D, H, HD = 2, 2048, 1024, 16, 64
NCORES = 8
HPC = 4               # heads per core
LQK = HPC * HD        # 256 local q (or k) rows
SCALE = HD ** -0.5    # 0.125

_CACHE = {}


def _build_program():
    import concourse.tile as tile
    from concourse import bacc, mybir

    F32 = mybir.dt.float32
    BF16 = mybir.dt.bfloat16
    Exp = mybir.ActivationFunctionType.Exp
    Mult = mybir.AluOpType.mult

    nc = bacc.Bacc("TRN2", target_bir_lowering=False, debug=False,
                   num_devices=NCORES)

    xT_d = nc.dram_tensor("xT", [D, N], BF16, kind="ExternalInput").ap()
    wqkT_d = nc.dram_tensor("wqkT", [D, 2 * LQK], BF16, kind="ExternalInput").ap()
    wvT_d = nc.dram_tensor("wvT", [D, LQK], BF16, kind="ExternalInput").ap()
    wprojT_d = nc.dram_tensor("wprojT", [LQK, D], BF16, kind="ExternalInput").ap()
    ident_d = nc.dram_tensor("ident", [128, 128], BF16, kind="ExternalInput").ap()
    out_d = nc.dram_tensor("out", [N, D], BF16, kind="ExternalOutput").ap()

    KT = D // 128        # 8 contraction tiles for qkv gemms
    NB = N // 512        # 4 seq blocks
    NT = N // 128        # 16 seq tiles

    with tile.TileContext(nc) as tc:
        with (
            nc.allow_low_precision(reason="bf16 matmul operands"),
            tc.tile_pool(name="const", bufs=1) as cpool,
            tc.tile_pool(name="w", bufs=1) as wpool,
            tc.tile_pool(name="x", bufs=1) as xpool,
            tc.tile_pool(name="qk", bufs=1) as qkpool,
            tc.tile_pool(name="vaug", bufs=1) as vapool,
            tc.tile_pool(name="ao", bufs=1) as aopool,
            tc.tile_pool(name="probs", bufs=18) as prpool,
            tc.tile_pool(name="small", bufs=2) as smpool,
            tc.tile_pool(name="psbig", bufs=2, space="PSUM") as psbig,
            tc.tile_pool(name="pspv", bufs=2, space="PSUM") as pspv,
            tc.tile_pool(name="pstr", bufs=1, space="PSUM") as pstr,
            tc.tile_pool(name="psgp", bufs=1, space="PSUM") as psgp,
        ):
            ones_f32 = cpool.tile([128, 128], F32)
            nc.vector.memset(ones_f32[:, :], 1.0)
            ident_sb = cpool.tile([128, 128], BF16)
            nc.sync.dma_start(out=ident_sb[:, :], in_=ident_d[:, :])

            # ---- input DMAs (kt-chunked so compute starts early) ----
            x_sb = xpool.tile([128, KT, N], BF16)
            wqk_sb = wpool.tile([128, KT, 2 * LQK], BF16)
            wv_sb = wpool.tile([128, KT, LQK], BF16)
            xT_r = xT_d.rearrange("(kt p) n -> p kt n", p=128)
            wqkT_r = wqkT_d.rearrange("(kt p) m -> p kt m", p=128)
            wvT_r = wvT_d.rearrange("(kt p) m -> p kt m", p=128)
            for kt in range(KT):
                nc.sync.dma_start(out=wqk_sb[:, kt, :], in_=wqkT_r[:, kt, :])
                nc.sync.dma_start(out=x_sb[:, kt, :], in_=xT_r[:, kt, :])
            for kt in range(KT):
                nc.sync.dma_start(out=wv_sb[:, kt, :], in_=wvT_r[:, kt, :])
            wproj_sb = wpool.tile([128, 2, D], BF16)
            nc.sync.dma_start(
                out=wproj_sb[:, :, :],
                in_=wprojT_d.rearrange("(kt p) o -> p kt o", p=128))

            # qk_sb m-tile layout: m=0: q heads 0,1 / m=1: q heads 2,3
            #                      m=2: k heads 0,1 / m=3: k heads 2,3
            qk_sb = qkpool.tile([128, 4, N], BF16)
            v_sb = vapool.tile([128, NT, HPC, HD + 1], BF16)
            ao_sb = aopool.tile([128, 2, N], BF16)

            # ---- filler generators: one matmul per yield ----
            def gen_qk(m, nb):
                wm = 0 if m < 2 else LQK          # q cols 0..255, k cols 256..511
                wcol = wm + (m % 2) * 128
                gp = psgp.tile([128, 512], F32, tag="gp")
                for kt in range(KT):
                    nc.tensor.matmul(
                        gp[:, :],
                        wqk_sb[:, kt, wcol:wcol + 128],
                        x_sb[:, kt, nb * 512:(nb + 1) * 512],
                        start=(kt == 0), stop=(kt == KT - 1),
                    )
                    yield
                nc.vector.tensor_copy(
                    qk_sb[:, m, nb * 512:(nb + 1) * 512], gp[:, :])

            def gen_v(st):
                gp = psgp.tile([128, 512], F32, tag="gp")
                for kt in range(KT):
                    nc.tensor.matmul(
                        gp[:, 0:LQK],
                        x_sb[:, kt, st * 128:(st + 1) * 128],
                        wv_sb[:, kt, :],
                        start=(kt == 0), stop=(kt == KT - 1),
                    )
                    yield
                nc.vector.tensor_copy(
                    v_sb[:, st, :, 0:HD],
                    gp[:, 0:LQK].rearrange("p (h d) -> p h d", h=HPC))
                nc.vector.tensor_copy(
                    v_sb[:, st, :, HD:HD + 1],
                    ones_f32[:, 0:HPC].rearrange("p (h c) -> p h c", c=1))

            outst_by_nt = {}

            def gen_proj(nt, ob):
                gp = psgp.tile([128, 512], F32, tag="gp")
                for kt2 in range(2):
                    nc.tensor.matmul(
                        gp[:, :],
                        ao_sb[:, kt2, nt * 128:(nt + 1) * 128],
                        wproj_sb[:, kt2, ob * 512:(ob + 1) * 512],
                        start=(kt2 == 0), stop=(kt2 == 1),
                    )
                    yield
                if ob == 0:
                    outst = smpool.tile([128, 1024], BF16, tag="outst")
                    outst_by_nt[nt] = outst
                else:
                    outst = outst_by_nt.pop(nt)
                nc.vector.tensor_copy(outst[:, ob * 512:(ob + 1) * 512], gp[:, :])
                if ob == 1:
                    nc.sync.dma_start(out=out_d[nt * 128:(nt + 1) * 128, :],
                                      in_=outst[:, :])

            queue = deque()

            def pump(n):
                while n > 0 and queue:
                    try:
                        next(queue[0])
                        n -= 1
                    except StopIteration:
                        queue.popleft()

            def pump_all():
                while queue:
                    pump(1 << 30)

            # ---- attention primitives (software-pipelined chains) ----
            def scores_one(h, qb, kk):
                """2 score matmuls + exp for key-tile pair kk; returns probs."""
                pi = (h % 2) * 64
                mq, mk = h // 2, 2 + h // 2
                qT = qk_sb[pi:pi + 64, mq, qb * 512:(qb + 1) * 512]
                sc = psbig.tile([128, 1024], F32, tag="big")
                pr = prpool.tile([128, 1024], BF16, tag="probs")
                for j in range(2):
                    kt = 2 * kk + j
                    kT = qk_sb[pi:pi + 64, mk, kt * 128:(kt + 1) * 128]
                    nc.tensor.matmul(
                        sc[:, j * 512:(j + 1) * 512], kT, qT,
                        start=True, stop=True)
                nc.scalar.activation(pr[:, :], sc[:, :], Exp, scale=SCALE)
                return pr

            def pvs_one(pv, h, kk, pr):
                for qs in range(4):
                    for j in range(2):
                        kt = 2 * kk + j
                        nc.tensor.matmul(
                            pv[:, qs, 0:HD + 1],
                            pr[:, j * 512 + qs * 128:j * 512 + (qs + 1) * 128],
                            v_sb[:, kt, h, :],
                            start=False,
                            stop=(kk == KT - 1 and j == 1),
                            skip_group_check=True,
                        )

            def norm_tail(h, qb, pv):
                """DVE: recip of denominators + per-partition normalize to
                bf16; PE: transpose back to [hd, q]; DVE: copy into ao_sb."""
                pi = (h % 2) * 64
                den = smpool.tile([128, 4], F32, tag="den")
                nc.vector.tensor_copy(den[:, :], pv[:, :, HD])
                rec = smpool.tile([128, 4], F32, tag="rec")
                nc.vector.reciprocal(rec[:, :], den[:, :])
                aos = smpool.tile([128, 4, HD], BF16, tag="aos")
                for qs in range(4):
                    nc.vector.tensor_scalar(
                        aos[:, qs, :], pv[:, qs, 0:HD], rec[:, qs:qs + 1],
                        None, Mult)
                pt = pstr.tile([64, 1024], BF16, tag="tr")
                for qs in range(4):
                    nc.tensor.matmul(
                        pt[:, qs * 128:(qs + 1) * 128], aos[:, qs, :],
                        ident_sb[:, :], is_transpose=True)
                nc.vector.tensor_copy(
                    ao_sb[pi:pi + 64, h // 2, qb * 512:(qb + 1) * 512],
                    pt[:, 0:512])

            # ---- schedule ----
            # prelude emitted directly through the (still idle) scores pool
            # with double buffering: k h0/h1 (all nb), q h0/h1 for qb0, v st0/1
            def pre_qk(ps, half, m, nb):
                wm = 0 if m < 2 else LQK
                wcol = wm + (m % 2) * 128
                for kt in range(KT):
                    nc.tensor.matmul(
                        ps[:, half * 512:(half + 1) * 512],
                        wqk_sb[:, kt, wcol:wcol + 128],
                        x_sb[:, kt, nb * 512:(nb + 1) * 512],
                        start=(kt == 0), stop=(kt == KT - 1),
                    )

            for nn in range(2):             # k h0/h1: nb pairs (0,1) and (2,3)
                ps = psbig.tile([128, 1024], F32, tag="big")
                for half in range(2):
                    pre_qk(ps, half, 2, nn * 2 + half)
                nc.vector.tensor_copy(
                    qk_sb[:, 2, nn * 1024:(nn + 1) * 1024], ps[:, :])
            ps = psbig.tile([128, 1024], F32, tag="big")
            pre_qk(ps, 0, 0, 0)             # q h0/h1 for qb0
            for kt in range(KT):            # v st0 into the second bank
                nc.tensor.matmul(
                    ps[:, 512:512 + LQK],
                    x_sb[:, kt, 0:128], wv_sb[:, kt, :],
                    start=(kt == 0), stop=(kt == KT - 1),
                )
            nc.vector.tensor_copy(qk_sb[:, 0, 0:512], ps[:, 0:512])
            nc.vector.tensor_copy(
                v_sb[:, 0, :, 0:HD],
                ps[:, 512:512 + LQK].rearrange("p (h d) -> p h d", h=HPC))
            nc.vector.tensor_copy(
                v_sb[:, 0, :, HD:HD + 1],
                ones_f32[:, 0:HPC].rearrange("p (h c) -> p h c", c=1))
            ps = psbig.tile([128, 1024], F32, tag="big")
            for kt in range(KT):            # v st1
                nc.tensor.matmul(
                    ps[:, 0:LQK],
                    x_sb[:, kt, 128:256], wv_sb[:, kt, :],
                    start=(kt == 0), stop=(kt == KT - 1),
                )
            nc.vector.tensor_copy(
                v_sb[:, 1, :, 0:HD],
                ps[:, 0:LQK].rearrange("p (h d) -> p h d", h=HPC))
            nc.vector.tensor_copy(
                v_sb[:, 1, :, HD:HD + 1],
                ones_f32[:, 0:HPC].rearrange("p (h c) -> p h c", c=1))

            # v must be complete before any PV consumes it (JIT v copies
            # race same-chain PV reads): all v gens are pumped during chain
            # 0's scores phase, front-loaded so the last copy lands a full
            # key-pair before the first PV.
            queue.extend([gen_v(st) for st in range(2, NT)])
            queue.append(gen_qk(1, 0))                          # q h2/h3 qb0
            queue.extend([gen_qk(3, nb) for nb in range(NB)])   # k h2/h3
            for nb in range(1, NB):
                queue.append(gen_qk(0, nb))
                queue.append(gen_qk(1, nb))

            chains = [(h, qb) for qb in range(NB) for h in range(HPC)]
            NC = len(chains)

            # chain 0 scores phase, v tiles as filler (front-loaded)
            prs = {0: []}
            for kk in range(KT):
                prs[0].append(scores_one(0, 0, kk))
                pump(16 if kk < KT - 1 else 0)

            # steady state: chain c's PVs interleaved with chain c+1's
            # scores/exps so ACT has no chain-boundary bubble
            for c in range(NC):
                h, qb = chains[c]
                nxt = chains[c + 1] if c + 1 < NC else None
                pv = pspv.tile([128, 4, 128], F32, tag="pv")
                nc.vector.memset(pv[:, :, :], 0.0)  # full-bank tile
                if nxt is not None and nxt[1] != qb:
                    pump(8)     # flush the next q-block's q tiles
                if nxt is not None:
                    prs[c + 1] = []
                # chain 0 must flush q(h23)@qb0 + all k(h23) (40 matmuls)
                # before chain 2's scores prefetch inside chain 1's PV phase;
                # 3rd chain of a q-block pre-pumps the boundary backlog
                fill = 5 if c == 0 else (3 if (c % HPC == 2) else 2)
                for kk in range(KT):
                    pvs_one(pv, h, kk, prs[c][kk])
                    if nxt is not None:
                        prs[c + 1].append(scores_one(nxt[0], nxt[1], kk))
                    pump(fill)
                del prs[c]
                norm_tail(h, qb, pv)
                if c % HPC == HPC - 1 and qb < NB - 1:
                    for nt in range(qb * 4, qb * 4 + 4):
                        queue.append(gen_proj(nt, 0))
                        queue.append(gen_proj(nt, 1))

            pump_all()
            for nt in range(12, 16):
                ps = psbig.tile([128, 1024], F32, tag="big")
                for ob in range(2):
                    for kt2 in range(2):
                        nc.tensor.matmul(
                            ps[:, ob * 512:(ob + 1) * 512],
                            ao_sb[:, kt2, nt * 128:(nt + 1) * 128],
                            wproj_sb[:, kt2, ob * 512:(ob + 1) * 512],
                            start=(kt2 == 0), stop=(kt2 == 1),
                        )
                outst = smpool.tile([128, D], BF16, tag="outbig")
                nc.vector.tensor_copy(outst[:, :], ps[:, :])
                nc.sync.dma_start(out=out_d[nt * 128:(nt + 1) * 128, :],
                                  in_=outst[:, :])

    nc.compile()
    return nc


def _get_program():
    if "nc" not in _CACHE:
        _CACHE["nc"] = _build_program()
    return _CACHE["nc"]


def _make_in_maps(x, w_qkv, w_proj):
    import ml_dtypes
    bf16 = ml_dtypes.bfloat16
    x = np.asarray(x, dtype=np.float32)
    w_qkv = np.asarray(w_qkv, dtype=np.float32)
    w_proj = np.asarray(w_proj, dtype=np.float32)
    ident = np.eye(128, dtype=np.float32).astype(bf16)
    xT = [np.ascontiguousarray(x[b].T).astype(bf16) for b in range(B)]
    in_maps = []
    for c in range(NCORES):
        b, hg = c // 4, c % 4
        rows = slice(hg * LQK, (hg + 1) * LQK)
        qk_rows = np.r_[np.arange(hg * LQK, (hg + 1) * LQK),
                        D + np.arange(hg * LQK, (hg + 1) * LQK)]
        in_maps.append({
            "xT": xT[b],
            "wqkT": np.ascontiguousarray(w_qkv[qk_rows, :].T).astype(bf16),
            "wvT": np.ascontiguousarray(
                w_qkv[2 * D + np.arange(hg * LQK, (hg + 1) * LQK), :].T).astype(bf16),
            "wprojT": np.ascontiguousarray(w_proj[:, rows].T).astype(bf16),
            "ident": ident,
        })
    return in_maps


def kernel(x, w_qkv, w_proj, b_proj, _return_results=False, _trace=False):
    from concourse import bass_utils

    nc = _get_program()
    in_maps = _make_in_maps(x, w_qkv, w_proj)
    res = bass_utils.run_bass_kernel_spmd(
        nc, in_maps, list(range(NCORES)), trace=_trace)
    partials = np.stack([res.results[c]["out"] for c in range(NCORES)])
    out = partials.reshape(B, 4, N, D).sum(axis=1, dtype=np.float32)
    out = out + np.asarray(b_proj, dtype=np.float32)[None, None, :]
    out = out.astype(np.float32)
    if _return_results:
        return out, res
    return out
